# revision 1
# baseline (speedup 1.0000x reference)
"""Trainium2 Bass kernel for nn_CausalSelfAttention_39685497815389.

Self-contained: host-side sharding/prep + Bass/Tile kernel + 8-core SPMD run.

Sharding: head-parallel. Core c owns heads {2c, 2c+1} = channel slice
[128c, 128c+128). All per-head work (proj, token-shift, LN, RoPE, causal
attention) is local. An AllToAll redistributes attention output yT from
head-sharded to token-sharded; each core then computes c_proj + residual
for its own 512-token slice. Host concatenates the 8 slices.

Compute domain is "transposed": activations live as [channel(partition),
token(free)] so the per-channel token-shift mix and LN scale/bias become
per-partition scalars and the token-shift itself is a free-dim offset.
"""

import numpy as np

import concourse.bacc as bacc
import concourse.tile as tile
import concourse.mybir as mybir
from concourse.bass_utils import run_bass_kernel_spmd

B, T, C, H, HN = 2, 2048, 1024, 16, 64
BT = B * T
N_CORES = 8
G = 512                 # token chunk size
NG = BT // G            # 8 chunks
ROPE_PARTIAL = 32
ROPE_THETA = 10000.0
LN_EPS = 1e-5
SCALE = 1.0 / 8.0       # 1/sqrt(HN)

F32 = mybir.dt.float32
F32R = mybir.dt.float32r
BF16 = mybir.dt.bfloat16
I32 = mybir.dt.int32
AF = mybir.ActivationFunctionType
OP = mybir.AluOpType

# consts tensor layout: [128, 652]
#   [:, 0:128]    ind2: block-diag 1/64
#   [:, 128:256]  pswap: rope pair-swap permutation
#   [:, 256:384]  identity
#   [0:4, 384:512]  indq4 (rstd broadcast select rows 0,1)
#   [0:4, 512:640]  indk4 (rstd broadcast select rows 2,3)
#   [:, 640:652]  per-partition cols:
COL_MIXQ, COL_OMQ, COL_MIXK, COL_OMK, COL_MIXV, COL_OMV = 0, 1, 2, 3, 4, 5
COL_V0H, COL_WQ, COL_BQ, COL_WK, COL_BK = 6, 7, 8, 9, 10
N_COLS = 12
CONST_W = 652


def _rope_tables():
    ang = (1.0 / ROPE_THETA) ** np.linspace(0.0, 1.0, ROPE_PARTIAL // 2,
                                            dtype=np.float64)
    ang = np.repeat(ang, 2)                                  # [32]
    theta = np.arange(T, dtype=np.float64)[:, None] * ang[None, :]  # [T, 32]
    cos = np.cos(theta)
    sign = np.where(np.arange(ROPE_PARTIAL) % 2 == 1, -1.0, 1.0)
    sin = np.sin(theta) * sign[None, :]
    # extend to [T, 64]: cos=1, sin=0 beyond partial
    cosf = np.concatenate([cos, np.ones((T, HN - ROPE_PARTIAL))], axis=1)
    sinf = np.concatenate([sin, np.zeros((T, HN - ROPE_PARTIAL))], axis=1)
    # transposed + tiled for 2 heads: [128, T]
    cosT = np.tile(cosf.T, (2, 1)).astype(np.float32)
    sinT = np.tile(sinf.T, (2, 1)).astype(np.float32)
    return np.ascontiguousarray(cosT), np.ascontiguousarray(sinT)


def _host_prep(inputs):
    x = np.asarray(inputs["x"], np.float32).reshape(BT, C)
    v1 = np.asarray(inputs["v1"], np.float32).reshape(BT, C)
    residual = np.asarray(inputs["residual"], np.float32).reshape(BT, C)
    Wq, Wk, Wv = (np.asarray(inputs[k], np.float32) for k in ("Wq", "Wk", "Wv"))
    Wproj = np.asarray(inputs["Wproj"], np.float32)
    lora_a = np.asarray(inputs["lora_a"], np.float32)
    lora_b = np.asarray(inputs["lora_b"], np.float32)
    v0 = np.asarray(inputs["v0"], np.float32).reshape(C)
    xq_mix = np.asarray(inputs["xq_mix"], np.float32).reshape(C)
    xk_mix = np.asarray(inputs["xk_mix"], np.float32).reshape(C)
    xv_mix = np.asarray(inputs["xv_mix"], np.float32).reshape(C)
    lnq_w = np.asarray(inputs["lnq_w"], np.float32)
    lnq_b = np.asarray(inputs["lnq_b"], np.float32)
    lnk_w = np.asarray(inputs["lnk_w"], np.float32)
    lnk_b = np.asarray(inputs["lnk_b"], np.float32)

    xT = np.ascontiguousarray(x.T)                    # [C, BT]
    v1T = np.ascontiguousarray(v1.T)                  # [C, BT]
    WprojT = np.ascontiguousarray(Wproj.T)            # [C, C]
    cosT, sinT = _rope_tables()

    # shared consts
    ind2 = np.zeros((128, 128), np.float32)
    for h in range(2):
        ind2[h * 64:(h + 1) * 64, h * 64:(h + 1) * 64] = 1.0 / 64.0
    pswap = np.zeros((128, 128), np.float32)
    for m in range(128):
        if m % 64 < ROPE_PARTIAL:
            pswap[m ^ 1, m] = 1.0
    ident = np.eye(128, dtype=np.float32)
    indq4 = np.zeros((4, 128), np.float32)
    indk4 = np.zeros((4, 128), np.float32)
    for p in range(128):
        indq4[p // 64, p] = 1.0
        indk4[2 + p // 64, p] = 1.0

    in_maps = []
    for c in range(N_CORES):
        S = slice(128 * c, 128 * c + 128)
        wcat = np.concatenate(
            [Wq[S].T, Wk[S].T, Wv[S].T, lora_a], axis=1)   # [1024, 416]
        cols = np.zeros((128, N_COLS), np.float32)
        cols[:, COL_MIXQ] = xq_mix[S]
        cols[:, COL_OMQ] = 1.0 - xq_mix[S]
        cols[:, COL_MIXK] = xk_mix[S]
        cols[:, COL_OMK] = 1.0 - xk_mix[S]
        cols[:, COL_MIXV] = xv_mix[S]
        cols[:, COL_OMV] = 1.0 - xv_mix[S]
        cols[:, COL_V0H] = 0.5 * v0[S]
        cols[:, COL_WQ] = np.tile(lnq_w, 2)
        cols[:, COL_BQ] = np.tile(lnq_b, 2)
        cols[:, COL_WK] = np.tile(lnk_w, 2)
        cols[:, COL_BK] = np.tile(lnk_b, 2)
        consts = np.zeros((128, CONST_W), np.float32)
        consts[:, 0:128] = ind2
        consts[:, 128:256] = pswap
        consts[:, 256:384] = ident
        consts[0:4, 384:512] = indq4
        consts[0:4, 512:640] = indk4
        consts[:, 640:640 + N_COLS] = cols
        in_maps.append({
            "xT": xT,
            "v1t": np.ascontiguousarray(v1T[S]),          # [128, BT]
            "wcat": np.ascontiguousarray(wcat),
            "lorab": np.ascontiguousarray(lora_b[:, S]),  # [32, 128]
            "wpt": WprojT,
            "resid": np.ascontiguousarray(residual[512 * c:512 * c + 512]),
            "costab": cosT,
            "sintab": sinT,
            "consts": consts,
        })
    return in_maps


def _build(dbg=False, sim=False, nocc=False, stop=None, repeat=1):
    nc = bacc.Bacc("TRN2", target_bir_lowering=False, debug=False,
                   enable_asserts=True,
                   num_devices=1 if sim else N_CORES)
    xT_d = nc.dram_tensor("xT", [C, BT], F32, kind="ExternalInput").ap()
    v1t_d = nc.dram_tensor("v1t", [128, BT], F32, kind="ExternalInput").ap()
    wcat_d = nc.dram_tensor("wcat", [C, 416], F32, kind="ExternalInput").ap()
    lorab_d = nc.dram_tensor("lorab", [32, 128], F32, kind="ExternalInput").ap()
    wpt_d = nc.dram_tensor("wpt", [C, C], F32, kind="ExternalInput").ap()
    resid_d = nc.dram_tensor("resid", [512, C], F32, kind="ExternalInput").ap()
    costab_d = nc.dram_tensor("costab", [128, T], F32, kind="ExternalInput").ap()
    sintab_d = nc.dram_tensor("sintab", [128, T], F32, kind="ExternalInput").ap()
    consts_d = nc.dram_tensor("consts", [128, CONST_W], F32,
                              kind="ExternalInput").ap()
    out_d = nc.dram_tensor("out", [512, C], F32, kind="ExternalOutput").ap()
    dbg_d = {}
    if dbg:
        for nm in ("dbg_qfin", "dbg_kfin", "dbg_vf", "dbg_qraw"):
            dbg_d[nm] = nc.dram_tensor(nm, [128, BT], F32,
                                       kind="ExternalOutput").ap()
        dbg_d["dbg_yt"] = nc.dram_tensor("dbg_yt", [128, BT], F32,
                                         kind="ExternalOutput").ap()

    with tile.TileContext(nc) as tc:
        with tc.tile_pool(name="const", bufs=1) as cpool, \
             tc.tile_pool(name="big", bufs=1) as big, \
             tc.tile_pool(name="st", bufs=1) as st, \
             tc.tile_pool(name="psA", bufs=1, space="PSUM") as psA, \
             tc.tile_pool(name="psB", bufs=1, space="PSUM") as psB, \
             tc.tile_pool(name="dram", bufs=1, space="DRAM") as dpool:

            # ---------- constants ----------
            consts = cpool.tile([128, CONST_W], F32R)
            nc.sync.dma_start(out=consts, in_=consts_d.bitcast(F32R))
            ind2 = consts[:, 0:128]
            pswap = consts[:, 128:256]
            ident = consts.bitcast(F32)[:, 256:384]
            indq4 = consts[0:4, 384:512]
            indk4 = consts[0:4, 512:640]

            def col(i):
                return consts.bitcast(F32)[:, 640 + i:641 + i]

            wcat_sb = []
            for j in range(8):
                w = cpool.tile([128, 416], F32R, tag=f"wc{j}", name=f"wc{j}")
                nc.sync.dma_start(out=w,
                                  in_=wcat_d.bitcast(F32R)[128 * j:128 * (j + 1), :])
                wcat_sb.append(w)
            lorab_sb = cpool.tile([32, 128], F32R)
            nc.sync.dma_start(out=lorab_sb, in_=lorab_d.bitcast(F32R))
            costab = cpool.tile([128, T], F32, tag="cost")
            nc.sync.dma_start(out=costab, in_=costab_d)
            sintab = cpool.tile([128, T], F32, tag="sint")
            nc.sync.dma_start(out=sintab, in_=sintab_d)

            # ---------- persistent activations ----------
            q_fin = big.tile([128, BT], F32R, tag="qfin")
            k_fin = big.tile([128, BT], F32R, tag="kfin")
            vaug = [big.tile([128, 32, 65], BF16, tag=f"vaug{h}", name=f"vaug{h}")
                    for h in range(2)]
            for h in range(2):
                nc.vector.memset(vaug[h][:, :, 64:65], 1.0)
            yT = [big.tile([64, BT], F32, tag=f"yt{h}", name=f"yt{h}")
                  for h in range(2)]
            carry = big.tile([128, 4], F32, tag="carry")  # cols: qraw,kraw,vg last col

            # ---------- main per-chunk pipeline ----------
            for rep_g in [(rep, g) for rep in range(repeat) for g in range(NG)]:
                _, g = rep_g
                tcols = slice(G * g, G * (g + 1))
                first = g % 4 == 0          # batch-boundary chunk
                tsl = slice(G * (g % 4), G * (g % 4 + 1))

                # --- projections ---
                ps_q = psA.tile([128, G], F32, tag="pq")
                ps_k = psA.tile([128, G], F32, tag="pk")
                ps_v = psA.tile([128, G], F32, tag="pv")
                ps_u = psA.tile([32, G], F32, tag="pu")
                for j in range(8):
                    xt = st.tile([128, G], F32R, tag="xs", bufs=4)
                    nc.sync.dma_start(
                        out=xt, in_=xT_d.bitcast(F32R)[128 * j:128 * (j + 1), tcols])
                    lw = wcat_sb[j]
                    nc.tensor.matmul(ps_q, lw[:, 0:128], xt,
                                     start=(j == 0), stop=(j == 7))
                    nc.tensor.matmul(ps_k, lw[:, 128:256], xt,
                                     start=(j == 0), stop=(j == 7))
                    nc.tensor.matmul(ps_v, lw[:, 256:384], xt,
                                     start=(j == 0), stop=(j == 7))
                    nc.tensor.matmul(ps_u, lw[:, 384:416], xt,
                                     start=(j == 0), stop=(j == 7))
                u_sb = st.tile([32, G], F32R, tag="us", bufs=2)
                nc.vector.tensor_copy(u_sb, ps_u)
                raw = {}
                for tn, ps in (("q", ps_q), ("k", ps_k)):
                    r = st.tile([128, G], F32, tag=f"raw{tn}", name=f"raw{tn}", bufs=2)
                    nc.vector.tensor_copy(r, ps)
                    raw[tn] = r
                if dbg:
                    nc.sync.dma_start(out=dbg_d["dbg_qraw"][:, tcols],
                                      in_=raw["q"])

                # --- value pipeline ---
                gps = psB.tile([128, G], F32, tag="misc")
                nc.tensor.matmul(gps, lorab_sb, u_sb, start=True, stop=True)
                th = st.tile([128, G], F32, tag="wA")
                nc.scalar.activation(out=th, in_=gps, func=AF.Tanh,
                                     scale=0.5, bias=col(COL_V0H))
                sig = st.tile([128, G], F32, tag="wB")
                nc.vector.tensor_scalar(sig, th, 0.5, 0.5, OP.mult, OP.add)
                v1tile = st.tile([128, G], F32, tag="wC")
                nc.sync.dma_start(out=v1tile, in_=v1t_d[:, tcols])
                dd = st.tile([128, G], F32, tag="wD")
                nc.vector.tensor_sub(dd, v1tile, ps_v)
                nc.vector.tensor_mul(dd, dd, sig)
                vg = st.tile([128, G], F32, tag="vg")
                nc.vector.tensor_add(vg, dd, ps_v)

                def shift_mix(src_tile, carry_col, mix_c, om_c, out_tile):
                    # out = om*src + mix*prev(src); prev col0 from carry
                    t1 = st.tile([128, G], F32, tag="t1")
                    nc.vector.tensor_scalar_mul(t1[:, 1:G], src_tile[:, 0:G - 1],
                                                mix_c)
                    if first:
                        nc.vector.tensor_scalar_mul(t1[:, 0:1],
                                                    src_tile[:, 0:1], mix_c)
                    else:
                        nc.vector.tensor_scalar_mul(t1[:, 0:1], carry_col, mix_c)
                    nc.vector.scalar_tensor_tensor(out_tile, src_tile, om_c, t1,
                                                   OP.mult, OP.add)
                    nc.vector.tensor_copy(carry_col, src_tile[:, G - 1:G])

                vf = st.tile([128, G], F32, tag="wA2")
                shift_mix(vg, carry[:, 2:3], col(COL_MIXV), col(COL_OMV), vf)
                if dbg:
                    nc.sync.dma_start(out=dbg_d["dbg_vf"][:, tcols], in_=vf)
                for i in range(4):
                    tp = psB.tile([128, 128], F32, tag="misc")
                    nc.tensor.transpose(tp, vf[:, 128 * i:128 * (i + 1)], ident)
                    ti = 4 * g + i
                    nc.vector.tensor_copy(vaug[0][:, ti, 0:64], tp[:, 0:64])
                    nc.vector.tensor_copy(vaug[1][:, ti, 0:64], tp[:, 64:128])

                # --- q/k pipeline ---
                vscr = dpool.tile([4, G], F32, tag="vscr", bufs=2)
                qs_t = {}
                mu_t = {}
                for ti, tn in enumerate(("q", "k")):
                    mix_c = col(COL_MIXQ if tn == "q" else COL_MIXK)
                    om_c = col(COL_OMQ if tn == "q" else COL_OMK)
                    qs = st.tile([128, G], F32R, tag=f"qs{tn}", name=f"qs{tn}", bufs=2)
                    shift_mix(raw[tn], carry[:, ti:ti + 1], mix_c, om_c, qs)
                    qs_t[tn] = qs
                    ps_mu = psB.tile([128, G], F32, tag="stat", bufs=2)
                    nc.tensor.matmul(ps_mu, ind2, qs, start=True, stop=True)
                    mu_t[tn] = ps_mu
                    q2 = st.tile([128, G], F32R, tag="wB2")
                    nc.scalar.activation(out=q2, in_=qs.bitcast(F32),
                                         func=AF.Square)
                    ps_m2 = psB.tile([128, G], F32, tag="stat", bufs=2)
                    nc.tensor.matmul(ps_m2, ind2, q2, start=True, stop=True)
                    mu2 = st.tile([128, G], F32, tag="wC2")
                    nc.scalar.activation(out=mu2, in_=ps_mu, func=AF.Square)
                    varb = st.tile([128, G], F32, tag="wD2")
                    nc.vector.scalar_tensor_tensor(varb, ps_m2, LN_EPS, mu2,
                                                   OP.add, OP.subtract)
                    r0 = 2 * ti
                    nc.sync.dma_start(out=vscr[r0:r0 + 1, :], in_=varb[0:1, :])
                    nc.sync.dma_start(out=vscr[r0 + 1:r0 + 2, :],
                                      in_=varb[64:65, :])

                # rsqrt via bit-trick + 3 Newton iterations on [128, 16]
                tiny = st.tile([128, 16], F32, tag="tinyv")
                nc.sync.dma_start(out=tiny,
                                  in_=vscr.rearrange("a (p f) -> (a p) f", f=16))
                t1i = st.tile([128, 16], I32, tag="tiny1")
                nc.vector.tensor_scalar(t1i, tiny.bitcast(I32), 1, None,
                                        OP.arith_shift_right)
                y0i = st.tile([128, 16], I32, tag="tiny2")
                nc.vector.tensor_scalar(y0i, t1i, 0, None, OP.bitwise_not)
                ycur = st.tile([128, 16], F32, tag="tiny3")
                nc.vector.tensor_scalar(ycur.bitcast(I32), y0i,
                                        0x5F3759DF + 1, None, OP.add)
                ysq = st.tile([128, 16], F32, tag="tiny4")
                yu = st.tile([128, 16], F32, tag="tiny5")
                for _ in range(3):
                    nc.scalar.activation(out=ysq, in_=ycur, func=AF.Square)
                    nc.vector.tensor_mul(ysq, tiny, ysq)
                    nc.vector.tensor_scalar(yu, ysq, -0.5, 1.5, OP.mult, OP.add)
                    nc.vector.tensor_mul(ycur, ycur, yu)
                rscr = dpool.tile([4, G], F32, tag="rscr", bufs=2)
                nc.sync.dma_start(out=rscr.rearrange("a (p f) -> (a p) f", f=16),
                                  in_=ycur)
                rstd4 = st.tile([4, G], F32R, tag="rstd4")
                nc.sync.dma_start(out=rstd4, in_=rscr.bitcast(F32R))

                for ti, tn in enumerate(("q", "k")):
                    ind4 = indq4 if tn == "q" else indk4
                    w_c = col(COL_WQ if tn == "q" else COL_WK)
                    b_c = col(COL_BQ if tn == "q" else COL_BK)
                    fin = q_fin if tn == "q" else k_fin
                    qs = qs_t[tn]
                    ps_rb = psB.tile([128, G], F32, tag="bc")
                    nc.tensor.matmul(ps_rb, ind4, rstd4, start=True, stop=True)
                    z1 = st.tile([128, G], F32, tag="wE")
                    nc.vector.scalar_tensor_tensor(z1, qs.bitcast(F32), 0.0,
                                                   mu_t[tn], OP.bypass,
                                                   OP.subtract)
                    nc.vector.scalar_tensor_tensor(z1, z1, w_c, ps_rb,
                                                   OP.mult, OP.mult)
                    z3 = st.tile([128, G], F32R, tag=f"z3{tn}", name=f"z3{tn}", bufs=2)
                    nc.vector.tensor_scalar(z3, z1, b_c, None, OP.add)
                    ps_zf = psB.tile([128, G], F32, tag="bc")
                    nc.tensor.matmul(ps_zf, pswap, z3, start=True, stop=True)
                    m1 = st.tile([128, G], F32, tag="wB3")
                    nc.vector.tensor_mul(m1, z3.bitcast(F32), costab[:, tsl])
                    m2r = st.tile([128, G], F32, tag="wC3")
                    nc.vector.scalar_tensor_tensor(m2r, ps_zf, 0.0,
                                                   sintab[:, tsl],
                                                   OP.bypass, OP.mult)
                    nc.vector.tensor_add(fin[:, tcols], m1, m2r)

            if dbg:
                nc.sync.dma_start(out=dbg_d["dbg_qfin"], in_=q_fin.bitcast(F32))
                nc.sync.dma_start(out=dbg_d["dbg_kfin"], in_=k_fin.bitcast(F32))

            # ---------- attention ----------
            for rep_b in ([(rep, b) for rep in range(repeat) for b in range(B)]
                          if stop != "qk" else []):
                _, b = rep_b
                base = T * b
                for h in range(2):
                    hr = slice(64 * h, 64 * (h + 1))
                    for qc in range(4):
                        qsl = slice(base + G * qc, base + G * (qc + 1))
                        y_ps = psB.tile([65, G], F32, tag="bc")
                        nj = 4 * qc + 4
                        for j in range(nj):
                            stp = psB.tile([128, G], F32, tag="stat", bufs=2)
                            ksl = slice(base + 128 * j, base + 128 * (j + 1))
                            nc.tensor.matmul(stp, k_fin[hr, ksl],
                                             q_fin[hr, qsl],
                                             start=True, stop=True)
                            pt = st.tile([128, G], BF16, tag="pt", bufs=3)
                            nc.scalar.activation(out=pt, in_=stp, func=AF.Exp,
                                                 scale=SCALE)
                            off = 128 * j - G * qc
                            if off >= 0:
                                nc.gpsimd.affine_select(
                                    out=pt, in_=pt, compare_op=OP.is_ge,
                                    fill=0.0, base=-off, channel_multiplier=-1,
                                    pattern=[[1, G]])
                            nc.tensor.matmul(y_ps, vaug[h][:, 16 * b + j, :],
                                             pt, start=(j == 0),
                                             stop=(j == nj - 1))
                        sscr = dpool.tile([1, G], F32, tag="sscr", bufs=4)
                        srow = st.tile([128, G], F32, tag="srow")
                        nc.scalar.activation(out=srow[64:65, :],
                                             in_=y_ps[64:65, :], func=AF.Copy)
                        nc.sync.dma_start(out=sscr, in_=srow[64:65, :])
                        s_b = st.tile([64, G], F32, tag="sb")
                        nc.sync.dma_start(
                            out=s_b, in_=sscr[0:1, :].broadcast_to([64, G]))
                        rb = st.tile([64, G], F32, tag="rb")
                        nc.vector.reciprocal_approx_fast(rb, s_b)
                        nc.vector.scalar_tensor_tensor(
                            yT[h][:, qsl], y_ps[0:64, :], 0.0, rb,
                            OP.bypass, OP.mult)
            if dbg:
                nc.sync.dma_start(out=dbg_d["dbg_yt"][0:64, :], in_=yT[0])
                nc.sync.dma_start(out=dbg_d["dbg_yt"][64:128, :], in_=yT[1])
            if stop in ("qk", "attn"):
                for i in range(4):
                    nc.sync.dma_start(
                        out=out_d[128 * i:128 * (i + 1), :],
                        in_=q_fin.bitcast(F32)[:, 1024 * i:1024 * (i + 1)])

            # ---------- AllToAll + c_proj ----------
            if stop in ("qk", "attn"):
                a2a_in = None
            else:
                a2a_in = dpool.tile([8, 128, G], F32, tag="a2ain")
            if a2a_in is not None:
                a2a_out = dpool.tile([8, 128, G], F32, tag="a2aout")
            for blk in range(8) if a2a_in is not None else []:
                nc.sync.dma_start(out=a2a_in[blk, 0:64, :],
                                  in_=yT[0][:, G * blk:G * (blk + 1)])
                nc.sync.dma_start(out=a2a_in[blk, 64:128, :],
                                  in_=yT[1][:, G * blk:G * (blk + 1)])
            if a2a_in is not None:
                if sim or nocc:
                    nc.sync.dma_start(out=a2a_out, in_=a2a_in)
                else:
                    nc.gpsimd.collective_compute(
                        "AllToAll", OP.bypass,
                        replica_groups=[list(range(N_CORES))],
                        ins=[a2a_in.opt()], outs=[a2a_out.opt()])

            for co2 in range(2) if a2a_in is not None else []:
                wp = []
                for cc in range(8):
                    w = st.tile([128, G], F32R, tag="wp", name="wp", bufs=8)
                    nc.sync.dma_start(
                        out=w, in_=wpt_d.bitcast(F32R)[128 * cc:128 * (cc + 1),
                                                       G * co2:G * (co2 + 1)])
                    wp.append(w)
                for tt in range(4):
                    ops = psB.tile([128, G], F32, tag="stat", bufs=2)
                    for cc in range(8):
                        ytf = st.tile([128, 128], F32R, tag="ytf", bufs=4)
                        nc.sync.dma_start(
                            out=ytf,
                            in_=a2a_out.bitcast(F32R)[cc, :,
                                                      128 * tt:128 * (tt + 1)])
                        nc.tensor.matmul(ops, ytf, wp[cc],
                                         start=(cc == 0), stop=(cc == 7))
                    res_sb = st.tile([128, G], F32, tag="res")
                    nc.sync.dma_start(
                        out=res_sb,
                        in_=resid_d[128 * tt:128 * (tt + 1),
                                    G * co2:G * (co2 + 1)])
                    o_sb = st.tile([128, G], F32, tag="osb")
                    nc.vector.tensor_add(o_sb, ops, res_sb)
                    nc.sync.dma_start(
                        out=out_d[128 * tt:128 * (tt + 1),
                                  G * co2:G * (co2 + 1)],
                        in_=o_sb)

    nc.compile()
    return nc


_CACHE = {}


def _get_nc(dbg=False):
    if dbg not in _CACHE:
        _CACHE[dbg] = _build(dbg)
    return _CACHE[dbg]


def kernel(_dbg=False, _results_hook=None, **inputs):
    in_maps = _host_prep(inputs)
    nc = _get_nc(_dbg)
    res = run_bass_kernel_spmd(nc, in_maps, core_ids=list(range(N_CORES)))
    if _results_hook is not None:
        _results_hook(res)
    out = np.concatenate([res.results[c]["out"] for c in range(N_CORES)],
                         axis=0)
    return out.reshape(B, T, C)



# revision 10
# speedup vs baseline: 6.0231x; 6.0231x over previous
"""Trainium2 Bass kernel for nn_CausalSelfAttention_39685497815389.

Self-contained: host-side sharding/prep + Bass/Tile kernel + 8-core SPMD run.

Wall-clock through the axon tunnel is transfer-dominated, so the design
minimizes host<->device bytes:
  - fp16 everywhere on the wire; residual add happens on host.
  - x and v1 are uploaded token-sliced (contiguous views); the device
    transposes them and redistributes (AllGather x, AllToAll v1).
  - each core only receives its own head-slice of Wq/Wk/Wv and column
    slice of Wproj; c_proj partials are combined with a ReduceScatter.
  - rope tables and all 0/1 constant matrices are generated on device
    (iota + range-reduced Sin activation).

Sharding: head-parallel. Core c owns heads {2c, 2c+1} = channel slice
[128c, 128c+128). All per-head work (proj, token-shift, LN, RoPE, causal
attention) is local. c_proj is computed as partial products over the
core's 128 channels for all tokens; ReduceScatter sums partials and
leaves core c with tokens [512c, 512c+512). Host concatenates slices
and adds the residual.
"""

import numpy as np

import jax

for _k, _v in (("jax_compilation_cache_dir", "/root/.jax_cache"),
               ("jax_persistent_cache_min_entry_size_bytes", 0),
               ("jax_persistent_cache_min_compile_time_secs", 0)):
    try:
        jax.config.update(_k, _v)
    except Exception:
        pass

import concourse.bacc as bacc
import concourse.tile as tile
import concourse.mybir as mybir
from concourse.bass_utils import run_bass_kernel_spmd

B, T, C, H, HN = 2, 2048, 1024, 16, 64
BT = B * T
N_CORES = 8
G = 512                 # token chunk size
NG = BT // G            # 8 chunks
ROPE_PARTIAL = 32
ROPE_THETA = 10000.0
LN_EPS = 1e-5
SCALE = 1.0 / 8.0       # 1/sqrt(HN)
TWO_PI = 2.0 * np.pi

F32 = mybir.dt.float32
F32R = mybir.dt.float32r
F16 = mybir.dt.float16
I32 = mybir.dt.int32
AF = mybir.ActivationFunctionType
OP = mybir.AluOpType

# ccol per-partition constants: [128, N_COLS] f32
COL_MIXQ, COL_OMQ, COL_MIXK, COL_OMK, COL_MIXV, COL_OMV = 0, 1, 2, 3, 4, 5
COL_V0H, COL_WQ, COL_BQ, COL_WK, COL_BK, COL_ANG, COL_SGN = 6, 7, 8, 9, 10, 11, 12
N_COLS = 13


def _ang_sgn():
    ang64 = np.zeros(64, np.float32)
    angf = (1.0 / ROPE_THETA) ** np.linspace(0.0, 1.0, ROPE_PARTIAL // 2,
                                             dtype=np.float64)
    ang64[:ROPE_PARTIAL] = np.repeat(angf, 2)
    sgn64 = np.ones(64, np.float32)
    sgn64[1:ROPE_PARTIAL:2] = -1.0
    return np.tile(ang64, 2), np.tile(sgn64, 2)


_ANG, _SGN = _ang_sgn()


def _host_prep(inputs):
    f16 = np.float16
    x = np.asarray(inputs["x"], np.float32).reshape(BT, C).astype(f16)
    v1 = np.asarray(inputs["v1"], np.float32).reshape(BT, C).astype(f16)
    Wq = np.asarray(inputs["Wq"], np.float32)
    Wk = np.asarray(inputs["Wk"], np.float32)
    Wv = np.asarray(inputs["Wv"], np.float32)
    wpf = np.asarray(inputs["Wproj"], np.float32).astype(f16)
    laf = np.ascontiguousarray(np.asarray(inputs["lora_a"], np.float32)
                               .astype(f16))
    lbf = np.asarray(inputs["lora_b"], np.float32).astype(f16)
    v0 = np.asarray(inputs["v0"], np.float32).reshape(C)
    xq_mix = np.asarray(inputs["xq_mix"], np.float32).reshape(C)
    xk_mix = np.asarray(inputs["xk_mix"], np.float32).reshape(C)
    xv_mix = np.asarray(inputs["xv_mix"], np.float32).reshape(C)
    lnq_w = np.tile(np.asarray(inputs["lnq_w"], np.float32), 2)
    lnq_b = np.tile(np.asarray(inputs["lnq_b"], np.float32), 2)
    lnk_w = np.tile(np.asarray(inputs["lnk_w"], np.float32), 2)
    lnk_b = np.tile(np.asarray(inputs["lnk_b"], np.float32), 2)

    in_maps = []
    for c in range(N_CORES):
        S = slice(128 * c, 128 * c + 128)
        TS = slice(G * c, G * c + G)
        wqkv = np.concatenate([Wq[S], Wk[S], Wv[S]], axis=0).astype(f16)
        cols = np.zeros((128, N_COLS), np.float32)
        cols[:, COL_MIXQ] = xq_mix[S]
        cols[:, COL_OMQ] = 1.0 - xq_mix[S]
        cols[:, COL_MIXK] = xk_mix[S]
        cols[:, COL_OMK] = 1.0 - xk_mix[S]
        cols[:, COL_MIXV] = xv_mix[S]
        cols[:, COL_OMV] = 1.0 - xv_mix[S]
        cols[:, COL_V0H] = 0.5 * v0[S]
        cols[:, COL_WQ] = lnq_w
        cols[:, COL_BQ] = lnq_b
        cols[:, COL_WK] = lnk_w
        cols[:, COL_BK] = lnk_b
        cols[:, COL_ANG] = _ANG
        cols[:, COL_SGN] = _SGN
        in_maps.append({
            "xs": x[TS],                                     # [G, C] f16
            "v1s": v1[TS],                                   # [G, C] f16
            "wqkv": wqkv,                                    # [384, C] f16
            "wp": np.ascontiguousarray(wpf[:, S]),           # [C, 128] f16
            "la": laf,                                       # [C, 32] f16
            "lb": np.ascontiguousarray(lbf[:, S]),           # [32, 128] f16
            "ccol": cols,                                    # [128, 13] f32
        })
    return in_maps


def _build():
    nc = bacc.Bacc("TRN2", target_bir_lowering=False, debug=False,
                   enable_asserts=True, num_devices=N_CORES)
    xs_d = nc.dram_tensor("xs", [G, C], F16, kind="ExternalInput").ap()
    v1s_d = nc.dram_tensor("v1s", [G, C], F16, kind="ExternalInput").ap()
    wqkv_d = nc.dram_tensor("wqkv", [384, C], F16, kind="ExternalInput").ap()
    wp_d = nc.dram_tensor("wp", [C, 128], F16, kind="ExternalInput").ap()
    la_d = nc.dram_tensor("la", [C, 32], F16, kind="ExternalInput").ap()
    lb_d = nc.dram_tensor("lb", [32, 128], F16, kind="ExternalInput").ap()
    ccol_d = nc.dram_tensor("ccol", [128, N_COLS], F32,
                            kind="ExternalInput").ap()
    out_d = nc.dram_tensor("out", [G, C], F16, kind="ExternalOutput").ap()

    RG = [list(range(N_CORES))]

    with tile.TileContext(nc) as tc:
        with tc.tile_pool(name="const", bufs=1) as cpool, \
             tc.tile_pool(name="big", bufs=1) as big, \
             tc.tile_pool(name="st", bufs=1) as st, \
             tc.tile_pool(name="psA", bufs=1, space="PSUM") as psA, \
             tc.tile_pool(name="psB", bufs=1, space="PSUM") as psB, \
             tc.tile_pool(name="dram", bufs=1, space="DRAM") as dpool:

            # ---------- per-partition constants ----------
            ccol = cpool.tile([128, N_COLS], F32)
            nc.sync.dma_start(out=ccol, in_=ccol_d)

            def col(i):
                return ccol[:, i:i + 1]

            # ---------- generated constant matrices ----------
            pidx = cpool.tile([128, 1], I32, tag="pidx")
            nc.gpsimd.iota(pidx, pattern=[[1, 1]], base=0, channel_multiplier=1)
            fidx = cpool.tile([128, 128], I32, tag="fidx")
            nc.gpsimd.iota(fidx, pattern=[[1, 128]], base=0,
                           channel_multiplier=0)
            pidx_f = cpool.tile([128, 1], F32, tag="pidxf")
            nc.vector.tensor_copy(pidx_f, pidx)
            fidx_f = cpool.tile([128, 128], F32, tag="fidxf")
            nc.vector.tensor_copy(fidx_f, fidx)

            # identity (f16): 1 where f == p
            ident16 = cpool.tile([128, 128], F16, tag="ident16")
            nc.vector.tensor_scalar(ident16, fidx_f, pidx_f, None, OP.is_equal)

            # ind2 (f16): 1/64 where f//64 == p//64
            fdiv = cpool.tile([128, 128], I32, tag="fdiv")
            nc.vector.tensor_scalar(fdiv, fidx, 6, None, OP.arith_shift_right)
            pdiv = cpool.tile([128, 1], I32, tag="pdiv")
            nc.vector.tensor_scalar(pdiv, pidx, 6, None, OP.arith_shift_right)
            fdiv_f = cpool.tile([128, 128], F32, tag="fdivf")
            nc.vector.tensor_copy(fdiv_f, fdiv)
            pdiv_f = cpool.tile([128, 1], F32, tag="pdivf")
            nc.vector.tensor_copy(pdiv_f, pdiv)
            ind2 = cpool.tile([128, 128], F16, tag="ind2")
            nc.vector.tensor_scalar(ind2, fdiv_f, pdiv_f, 1.0 / 64.0,
                                    OP.is_equal, OP.mult)

            # pswap (f16): 1 where f == p^1 (pair swap; rows >=32 are
            # harmless because sintab is 0 there)
            pm2 = cpool.tile([128, 1], I32, tag="pm2")
            nc.vector.tensor_scalar(pm2, pidx, 1, None, OP.bitwise_and)
            tgt = cpool.tile([128, 1], I32, tag="tgt")
            nc.vector.tensor_scalar(tgt, pm2, -2, 1, OP.mult, OP.add)
            nc.vector.tensor_tensor(tgt, tgt, pidx, OP.add)
            tgt_f = cpool.tile([128, 1], F32, tag="tgtf")
            nc.vector.tensor_copy(tgt_f, tgt)
            pswap = cpool.tile([128, 128], F16, tag="pswap")
            nc.vector.tensor_scalar(pswap, fidx_f, tgt_f, None, OP.is_equal)

            # ---------- rope tables (f32 [128, T]) ----------
            costab = cpool.tile([128, T], F32, tag="cost")
            sintab = cpool.tile([128, T], F32, tag="sint")
            halfpi = cpool.tile([128, 1], F32, tag="halfpi")
            nc.vector.memset(halfpi, np.pi / 2.0)
            zerob = cpool.tile([128, 1], F32, tag="zerob")
            nc.vector.memset(zerob, 0.0)
            for cc in range(T // G):
                csl = slice(G * cc, G * (cc + 1))
                ti32 = st.tile([128, G], I32, tag="ti32", bufs=2)
                nc.gpsimd.iota(ti32, pattern=[[1, G]], base=G * cc,
                               channel_multiplier=0)
                th = st.tile([128, G], F32, tag="ropeth", bufs=2)
                nc.vector.tensor_copy(th, ti32)
                nc.vector.tensor_scalar_mul(th, th, col(COL_ANG))
                for tab, shift, bias in ((sintab, 0.0, zerob),
                                         (costab, 0.25, halfpi)):
                    uu = st.tile([128, G], F32, tag="ropeuu")
                    nc.vector.tensor_scalar(uu, th, 1.0 / TWO_PI, shift,
                                            OP.mult, OP.add)
                    ki = st.tile([128, G], I32, tag="ropeki")
                    nc.vector.tensor_copy(ki, uu)   # rounds to nearest
                    kf = st.tile([128, G], F32, tag="ropekf")
                    nc.vector.tensor_copy(kf, ki)
                    red = st.tile([128, G], F32, tag="ropered")
                    nc.vector.scalar_tensor_tensor(red, kf, -TWO_PI, th,
                                                   OP.mult, OP.add)
                    nc.scalar.activation(out=tab[:, csl], in_=red,
                                         func=AF.Sin, bias=bias[:, 0:1])
            nc.vector.tensor_scalar_mul(sintab, sintab, col(COL_SGN))

            # ---------- weights: load + transpose ----------
            wqt = cpool.tile([128, C], F16, tag="wqt")
            wkt = cpool.tile([128, C], F16, tag="wkt")
            wvt = cpool.tile([128, C], F16, tag="wvt")
            wpTh = [cpool.tile([64, C], F16, tag=f"wpT{h}", name=f"wpT{h}")
                    for h in range(2)]
            la_sb = cpool.tile([128, 256], F16, tag="la")
            lb_sb = cpool.tile([32, 128], F16, tag="lb")
            nc.sync.dma_start(out=lb_sb, in_=lb_d)
            for j in range(8):
                nc.sync.dma_start(out=la_sb[:, 32 * j:32 * (j + 1)],
                                  in_=la_d[128 * j:128 * (j + 1), :])
            for wt, row0 in ((wqt, 0), (wkt, 128), (wvt, 256)):
                wraw = st.tile([128, C], F16, tag="wraw", bufs=2)
                nc.sync.dma_start(out=wraw,
                                  in_=wqkv_d[row0:row0 + 128, :])
                for j in range(8):
                    tp = psB.tile([128, 128], F16, tag="tp", bufs=1)
                    nc.tensor.transpose(tp, wraw[:, 128 * j:128 * (j + 1)],
                                        ident16)
                    nc.vector.tensor_copy(wt[:, 128 * j:128 * (j + 1)], tp)
            for m in range(8):
                wpraw = st.tile([128, 128], F16, tag="wpraw", bufs=2)
                nc.sync.dma_start(out=wpraw,
                                  in_=wp_d[128 * m:128 * (m + 1), :])
                tp = psB.tile([128, 128], F16, tag="tp", bufs=1)
                nc.tensor.transpose(tp, wpraw, ident16)
                wpscr = st.tile([128, 128], F16, tag="wpscr", bufs=2)
                nc.vector.tensor_copy(wpscr, tp)
                nc.vector.tensor_copy(wpTh[0][:, 128 * m:128 * (m + 1)],
                                      wpscr[0:64, :])
                nc.sync.dma_start(out=wpTh[1][:, 128 * m:128 * (m + 1)],
                                  in_=wpscr[64:128, :])

            # ---------- ingest x/v1: transpose + collectives ----------
            ag_in = dpool.tile([8, 128, G], F16, tag="agin")
            a2a_in = dpool.tile([8, 128, G], F16, tag="a2ain")
            for src_d, dst in ((xs_d, ag_in), (v1s_d, a2a_in)):
                xsb = []
                for a in range(4):
                    xa = st.tile([128, C], F16, tag=f"xsb{a}", name=f"xsb{a}",
                                 bufs=2)
                    nc.sync.dma_start(out=xa,
                                      in_=src_d[128 * a:128 * (a + 1), :])
                    xsb.append(xa)
                for j in range(8):
                    xtj = st.tile([128, G], F16, tag="xtj", bufs=3)
                    for a in range(4):
                        tp = psB.tile([128, 128], F16, tag="tp", bufs=1)
                        nc.tensor.transpose(
                            tp, xsb[a][:, 128 * j:128 * (j + 1)], ident16)
                        nc.vector.tensor_copy(
                            xtj[:, 128 * a:128 * (a + 1)], tp)
                    nc.sync.dma_start(out=dst[j], in_=xtj)
            ag_out = dpool.tile([8, 8, 128, G], F16, tag="agout")
            nc.gpsimd.collective_compute(
                "AllGather", OP.bypass, replica_groups=RG,
                ins=[ag_in.opt()], outs=[ag_out.opt()])
            a2a_out = dpool.tile([8, 128, G], F16, tag="a2aout")
            nc.gpsimd.collective_compute(
                "AllToAll", OP.bypass, replica_groups=RG,
                ins=[a2a_in.opt()], outs=[a2a_out.opt()])

            # ---------- persistent activations ----------
            q_fin = big.tile([128, BT], F16, tag="qfin")
            k_fin = big.tile([128, BT], F16, tag="kfin")
            vaug = [big.tile([128, 32, 65], F16, tag=f"vaug{h}",
                             name=f"vaug{h}") for h in range(2)]
            for h in range(2):
                nc.vector.memset(vaug[h][:, :, 64:65], 1.0)
            yT = [big.tile([64, BT], F16, tag=f"yt{h}", name=f"yt{h}")
                  for h in range(2)]
            carry = big.tile([128, 4], F32, tag="carry")

            # ---------- main per-chunk pipeline ----------
            for g in range(NG):
                first = g % 4 == 0          # batch-boundary chunk
                tcols = slice(G * g, G * (g + 1))
                tsl = slice(G * (g % 4), G * (g % 4 + 1))

                # --- projections ---
                ps_q = psA.tile([128, G], F32, tag="pq")
                ps_k = psA.tile([128, G], F32, tag="pk")
                ps_v = psA.tile([128, G], F32, tag="pv")
                ps_u = psA.tile([32, G], F32, tag="pu")
                for j in range(8):
                    xt = st.tile([128, G], F16, tag="xs", bufs=4)
                    nc.sync.dma_start(out=xt, in_=ag_out[g, j])
                    nc.tensor.matmul(ps_q, wqt[:, 128 * j:128 * (j + 1)], xt,
                                     start=(j == 0), stop=(j == 7))
                    nc.tensor.matmul(ps_k, wkt[:, 128 * j:128 * (j + 1)], xt,
                                     start=(j == 0), stop=(j == 7))
                    nc.tensor.matmul(ps_v, wvt[:, 128 * j:128 * (j + 1)], xt,
                                     start=(j == 0), stop=(j == 7))
                    nc.tensor.matmul(ps_u, la_sb[:, 32 * j:32 * (j + 1)], xt,
                                     start=(j == 0), stop=(j == 7))
                u_sb = st.tile([32, G], F16, tag="us", bufs=2)
                nc.vector.tensor_copy(u_sb, ps_u)
                raw = {}
                for tn, ps in (("q", ps_q), ("k", ps_k)):
                    r = st.tile([128, G], F32, tag=f"raw{tn}", name=f"raw{tn}",
                                bufs=2)
                    nc.vector.tensor_copy(r, ps)
                    raw[tn] = r

                # --- value pipeline ---
                gps = psB.tile([128, G], F32, tag="stat", bufs=2)
                nc.tensor.matmul(gps, lb_sb, u_sb, start=True, stop=True)
                th_t = st.tile([128, G], F32, tag="wA")
                nc.scalar.activation(out=th_t, in_=gps, func=AF.Tanh,
                                     scale=0.5, bias=col(COL_V0H))
                sig = st.tile([128, G], F32, tag="wB")
                nc.vector.tensor_scalar(sig, th_t, 0.5, 0.5, OP.mult, OP.add)
                v1t16 = st.tile([128, G], F16, tag="v1a", bufs=2)
                nc.sync.dma_start(out=v1t16, in_=a2a_out[g])
                v1tile = st.tile([128, G], F32, tag="wC")
                nc.vector.tensor_copy(v1tile, v1t16)
                dd = st.tile([128, G], F32, tag="wD")
                nc.vector.tensor_sub(dd, v1tile, ps_v)
                nc.vector.tensor_mul(dd, dd, sig)
                vg = st.tile([128, G], F32, tag="vg")
                nc.vector.tensor_add(vg, dd, ps_v)

                def shift_mix(src_tile, carry_col, mix_c, om_c, out_tile):
                    # out = om*src + mix*prev(src); prev col0 from carry
                    t1 = st.tile([128, G], F32, tag="t1")
                    nc.vector.tensor_scalar_mul(t1[:, 1:G],
                                                src_tile[:, 0:G - 1], mix_c)
                    if first:
                        nc.vector.tensor_scalar_mul(t1[:, 0:1],
                                                    src_tile[:, 0:1], mix_c)
                    else:
                        nc.vector.tensor_scalar_mul(t1[:, 0:1], carry_col,
                                                    mix_c)
                    nc.vector.scalar_tensor_tensor(out_tile, src_tile, om_c,
                                                   t1, OP.mult, OP.add)
                    nc.vector.tensor_copy(carry_col, src_tile[:, G - 1:G])

                vf = st.tile([128, G], F32, tag="wA2")
                shift_mix(vg, carry[:, 2:3], col(COL_MIXV), col(COL_OMV), vf)
                vf16 = st.tile([128, G], F16, tag="vf16")
                nc.vector.tensor_copy(vf16, vf)
                for i in range(4):
                    tp = psB.tile([128, 128], F16, tag="tp", bufs=1)
                    nc.tensor.transpose(tp, vf16[:, 128 * i:128 * (i + 1)],
                                        ident16)
                    ti = 4 * g + i
                    nc.vector.tensor_copy(vaug[0][:, ti, 0:64], tp[:, 0:64])
                    nc.vector.tensor_copy(vaug[1][:, ti, 0:64], tp[:, 64:128])

                # --- q/k pipeline ---
                for ti, tn in enumerate(("q", "k")):
                    mix_c = col(COL_MIXQ if tn == "q" else COL_MIXK)
                    om_c = col(COL_OMQ if tn == "q" else COL_OMK)
                    w_c = col(COL_WQ if tn == "q" else COL_WK)
                    b_c = col(COL_BQ if tn == "q" else COL_BK)
                    fin = q_fin if tn == "q" else k_fin

                    qs = st.tile([128, G], F32, tag=f"qs{tn}", name=f"qs{tn}",
                                 bufs=2)
                    shift_mix(raw[tn], carry[:, ti:ti + 1], mix_c, om_c, qs)
                    qs16 = st.tile([128, G], F16, tag="qs16", bufs=2)
                    nc.vector.tensor_copy(qs16, qs)
                    ps_mu = psB.tile([128, G], F32, tag="stat", bufs=2)
                    nc.tensor.matmul(ps_mu, ind2, qs16, start=True, stop=True)
                    q2 = st.tile([128, G], F16, tag="wB2")
                    nc.scalar.activation(out=q2, in_=qs, func=AF.Square)
                    ps_m2 = psB.tile([128, G], F32, tag="stat", bufs=2)
                    nc.tensor.matmul(ps_m2, ind2, q2, start=True, stop=True)
                    mu2 = st.tile([128, G], F32, tag="wC2")
                    nc.scalar.activation(out=mu2, in_=ps_mu, func=AF.Square)
                    varb = st.tile([128, G], F32, tag="wD2")
                    nc.vector.scalar_tensor_tensor(varb, ps_m2, LN_EPS, mu2,
                                                   OP.add, OP.subtract)
                    sd = st.tile([128, G], F32, tag="wE2")
                    nc.scalar.activation(out=sd, in_=varb, func=AF.Sqrt)
                    rstd = st.tile([128, G], F32, tag="wF2")
                    nc.vector.reciprocal(rstd, sd)

                    z1 = st.tile([128, G], F32, tag="wE")
                    nc.vector.scalar_tensor_tensor(z1, qs, 0.0, ps_mu,
                                                   OP.bypass, OP.subtract)
                    nc.vector.scalar_tensor_tensor(z1, z1, w_c, rstd,
                                                   OP.mult, OP.mult)
                    z3 = st.tile([128, G], F32, tag=f"z3{tn}", name=f"z3{tn}",
                                 bufs=2)
                    nc.vector.tensor_scalar(z3, z1, b_c, None, OP.add)
                    z316 = st.tile([128, G], F16, tag="z316", bufs=2)
                    nc.vector.tensor_copy(z316, z3)
                    ps_zf = psB.tile([128, G], F32, tag="stat", bufs=2)
                    nc.tensor.matmul(ps_zf, pswap, z316, start=True, stop=True)
                    m1 = st.tile([128, G], F32, tag="wB3")
                    nc.vector.tensor_mul(m1, z3, costab[:, tsl])
                    m2r = st.tile([128, G], F32, tag="wC3")
                    nc.vector.scalar_tensor_tensor(m2r, ps_zf, 0.0,
                                                   sintab[:, tsl],
                                                   OP.bypass, OP.mult)
                    nc.vector.tensor_add(fin[:, tcols], m1, m2r)

            # ---------- attention ----------
            for b in range(B):
                base = T * b
                for h in range(2):
                    hr = slice(64 * h, 64 * (h + 1))
                    for qc in range(4):
                        qsl = slice(base + G * qc, base + G * (qc + 1))
                        y_ps = psB.tile([65, G], F32, tag="bc")
                        nj = 4 * qc + 4
                        for j in range(nj):
                            stp = psB.tile([128, G], F32, tag="stat", bufs=2)
                            ksl = slice(base + 128 * j, base + 128 * (j + 1))
                            nc.tensor.matmul(stp, k_fin[hr, ksl],
                                             q_fin[hr, qsl],
                                             start=True, stop=True)
                            pt = st.tile([128, G], F16, tag="pt", bufs=3)
                            nc.scalar.activation(out=pt, in_=stp, func=AF.Exp,
                                                 scale=SCALE)
                            off = 128 * j - G * qc
                            if off >= 0:
                                nc.gpsimd.affine_select(
                                    out=pt, in_=pt, compare_op=OP.is_ge,
                                    fill=0.0, base=-off, channel_multiplier=-1,
                                    pattern=[[1, G]])
                            nc.tensor.matmul(y_ps, vaug[h][:, 16 * b + j, :],
                                             pt, start=(j == 0),
                                             stop=(j == nj - 1))
                        sscr = dpool.tile([1, G], F32, tag="sscr", bufs=4)
                        srow = st.tile([128, G], F32, tag="srow")
                        nc.scalar.activation(out=srow[64:65, :],
                                             in_=y_ps[64:65, :], func=AF.Copy)
                        nc.sync.dma_start(out=sscr, in_=srow[64:65, :])
                        s_b = st.tile([64, G], F32, tag="sb")
                        nc.sync.dma_start(
                            out=s_b, in_=sscr[0:1, :].broadcast_to([64, G]))
                        rb = st.tile([64, G], F32, tag="rb")
                        nc.vector.reciprocal(rb, s_b)
                        nc.vector.scalar_tensor_tensor(
                            yT[h][:, qsl], y_ps[0:64, :], 0.0, rb,
                            OP.bypass, OP.mult)

            # ---------- c_proj partials + ReduceScatter ----------
            rs_in = dpool.tile([8, G, C], F16, tag="rsin")
            for tt in range(32):
                ts128 = slice(128 * tt, 128 * (tt + 1))
                for half in range(2):
                    csl = slice(512 * half, 512 * (half + 1))
                    ps_o = psB.tile([128, G], F32, tag="stat", bufs=2)
                    nc.tensor.matmul(ps_o, yT[0][:, ts128], wpTh[0][:, csl],
                                     start=True, stop=False)
                    nc.tensor.matmul(ps_o, yT[1][:, ts128], wpTh[1][:, csl],
                                     start=False, stop=True)
                    ob = st.tile([128, G], F16, tag="ob", bufs=3)
                    nc.vector.tensor_copy(ob, ps_o)
                    nc.sync.dma_start(
                        out=rs_in[tt // 4, 128 * (tt % 4):128 * (tt % 4 + 1),
                                  csl],
                        in_=ob)
            rs_out = dpool.tile([G, C], F16, tag="rsout")
            nc.gpsimd.collective_compute(
                "ReduceScatter", OP.add, replica_groups=RG,
                ins=[rs_in.opt()], outs=[rs_out.opt()])
            nc.sync.dma_start(out=out_d, in_=rs_out)

    nc.compile()
    return nc


_CACHE = {}


def _get_nc():
    if "nc" not in _CACHE:
        _CACHE["nc"] = _build()
    return _CACHE["nc"]


def kernel(_results_hook=None, **inputs):
    in_maps = _host_prep(inputs)
    nc = _get_nc()
    res = run_bass_kernel_spmd(nc, in_maps, core_ids=list(range(N_CORES)))
    if _results_hook is not None:
        _results_hook(res)
    y = np.concatenate([res.results[c]["out"] for c in range(N_CORES)],
                       axis=0).astype(np.float32)
    out = np.asarray(inputs["residual"], np.float32).reshape(BT, C) + y
    return out.reshape(B, T, C)


# revision 11
# speedup vs baseline: 7.3777x; 1.2249x over previous
"""Trainium2 Bass kernel for nn_CausalSelfAttention_39685497815389.

Self-contained: host-side sharding/prep + Bass/Tile kernel + 8-core SPMD run.

Wall-clock through the axon tunnel is transfer-dominated, so the design
minimizes host<->device bytes:
  - fp16 everywhere on the wire; residual add happens on host.
  - x and v1 are uploaded token-sliced (contiguous views); the device
    transposes them and redistributes (AllGather x, AllToAll v1).
  - each core only receives its own head-slice of Wq/Wk/Wv and column
    slice of Wproj; c_proj partials are combined with a ReduceScatter.
  - rope tables and all 0/1 constant matrices are generated on device
    (iota + range-reduced Sin activation).

Sharding: head-parallel. Core c owns heads {2c, 2c+1} = channel slice
[128c, 128c+128). All per-head work (proj, token-shift, LN, RoPE, causal
attention) is local. c_proj is computed as partial products over the
core's 128 channels for all tokens; ReduceScatter sums partials and
leaves core c with tokens [512c, 512c+512). Host concatenates slices
and adds the residual.
"""

import numpy as np

import jax

for _k, _v in (("jax_compilation_cache_dir", "/root/.jax_cache"),
               ("jax_persistent_cache_min_entry_size_bytes", 0),
               ("jax_persistent_cache_min_compile_time_secs", 0)):
    try:
        jax.config.update(_k, _v)
    except Exception:
        pass

import concourse.bacc as bacc
import concourse.tile as tile
import concourse.mybir as mybir
from concourse.bass_utils import run_bass_kernel_spmd

B, T, C, H, HN = 2, 2048, 1024, 16, 64
BT = B * T
N_CORES = 8
G = 512                 # token chunk size
NG = BT // G            # 8 chunks
ROPE_PARTIAL = 32
ROPE_THETA = 10000.0
LN_EPS = 1e-5
SCALE = 1.0 / 8.0       # 1/sqrt(HN)
TWO_PI = 2.0 * np.pi

F32 = mybir.dt.float32
F32R = mybir.dt.float32r
F16 = mybir.dt.float16
F8 = mybir.dt.float8e4
I32 = mybir.dt.int32
NP_F8 = mybir.dt.np(mybir.dt.float8e4)
FP8_ACT = True     # x, v1 on the wire in fp8
FP8_W = True       # wqkv, wp, la, lb on the wire in fp8
AF = mybir.ActivationFunctionType
OP = mybir.AluOpType

# ccol per-partition constants: [128, N_COLS] f32
COL_MIXQ, COL_OMQ, COL_MIXK, COL_OMK, COL_MIXV, COL_OMV = 0, 1, 2, 3, 4, 5
COL_V0H, COL_WQ, COL_BQ, COL_WK, COL_BK, COL_ANG, COL_SGN = 6, 7, 8, 9, 10, 11, 12
N_COLS = 13


def _ang_sgn():
    ang64 = np.zeros(64, np.float32)
    angf = (1.0 / ROPE_THETA) ** np.linspace(0.0, 1.0, ROPE_PARTIAL // 2,
                                             dtype=np.float64)
    ang64[:ROPE_PARTIAL] = np.repeat(angf, 2)
    sgn64 = np.ones(64, np.float32)
    sgn64[1:ROPE_PARTIAL:2] = -1.0
    return np.tile(ang64, 2), np.tile(sgn64, 2)


_ANG, _SGN = _ang_sgn()


def _host_prep(inputs):
    f16 = np.float16
    act_dt = NP_F8 if FP8_ACT else f16
    w_dt = NP_F8 if FP8_W else f16
    x = np.asarray(inputs["x"], np.float32).reshape(BT, C).astype(act_dt)
    v1 = np.asarray(inputs["v1"], np.float32).reshape(BT, C).astype(act_dt)
    Wq = np.asarray(inputs["Wq"], np.float32)
    Wk = np.asarray(inputs["Wk"], np.float32)
    Wv = np.asarray(inputs["Wv"], np.float32)
    wpf = np.asarray(inputs["Wproj"], np.float32).astype(w_dt)
    laf = np.ascontiguousarray(np.asarray(inputs["lora_a"], np.float32)
                               .astype(w_dt))
    lbf = np.asarray(inputs["lora_b"], np.float32).astype(w_dt)
    v0 = np.asarray(inputs["v0"], np.float32).reshape(C)
    xq_mix = np.asarray(inputs["xq_mix"], np.float32).reshape(C)
    xk_mix = np.asarray(inputs["xk_mix"], np.float32).reshape(C)
    xv_mix = np.asarray(inputs["xv_mix"], np.float32).reshape(C)
    lnq_w = np.tile(np.asarray(inputs["lnq_w"], np.float32), 2)
    lnq_b = np.tile(np.asarray(inputs["lnq_b"], np.float32), 2)
    lnk_w = np.tile(np.asarray(inputs["lnk_w"], np.float32), 2)
    lnk_b = np.tile(np.asarray(inputs["lnk_b"], np.float32), 2)

    in_maps = []
    for c in range(N_CORES):
        S = slice(128 * c, 128 * c + 128)
        TS = slice(G * c, G * c + G)
        wqkv = np.concatenate([Wq[S], Wk[S], Wv[S]], axis=0).astype(w_dt)
        cols = np.zeros((128, N_COLS), np.float32)
        cols[:, COL_MIXQ] = xq_mix[S]
        cols[:, COL_OMQ] = 1.0 - xq_mix[S]
        cols[:, COL_MIXK] = xk_mix[S]
        cols[:, COL_OMK] = 1.0 - xk_mix[S]
        cols[:, COL_MIXV] = xv_mix[S]
        cols[:, COL_OMV] = 1.0 - xv_mix[S]
        cols[:, COL_V0H] = 0.5 * v0[S]
        cols[:, COL_WQ] = lnq_w
        cols[:, COL_BQ] = lnq_b
        cols[:, COL_WK] = lnk_w
        cols[:, COL_BK] = lnk_b
        cols[:, COL_ANG] = _ANG
        cols[:, COL_SGN] = _SGN
        in_maps.append({
            "xs": x[TS],                                     # [G, C] f16
            "v1s": v1[TS],                                   # [G, C] f16
            "wqkv": wqkv,                                    # [384, C] f16
            "wp": np.ascontiguousarray(wpf[:, S]),           # [C, 128] f16
            "la": laf,                                       # [C, 32] f16
            "lb": np.ascontiguousarray(lbf[:, S]),           # [32, 128] f16
            "ccol": cols,                                    # [128, 13] f32
        })
    return in_maps


def _build():
    nc = bacc.Bacc("TRN2", target_bir_lowering=False, debug=False,
                   enable_asserts=True, num_devices=N_CORES)
    DT_ACT = F8 if FP8_ACT else F16
    DT_W = F8 if FP8_W else F16
    xs_d = nc.dram_tensor("xs", [G, C], DT_ACT, kind="ExternalInput").ap()
    v1s_d = nc.dram_tensor("v1s", [G, C], DT_ACT, kind="ExternalInput").ap()
    wqkv_d = nc.dram_tensor("wqkv", [384, C], DT_W, kind="ExternalInput").ap()
    wp_d = nc.dram_tensor("wp", [C, 128], DT_W, kind="ExternalInput").ap()
    la_d = nc.dram_tensor("la", [C, 32], DT_W, kind="ExternalInput").ap()
    lb_d = nc.dram_tensor("lb", [32, 128], DT_W, kind="ExternalInput").ap()
    ccol_d = nc.dram_tensor("ccol", [128, N_COLS], F32,
                            kind="ExternalInput").ap()
    out_d = nc.dram_tensor("out", [G, C], F16, kind="ExternalOutput").ap()

    RG = [list(range(N_CORES))]

    with tile.TileContext(nc) as tc:
        with tc.tile_pool(name="const", bufs=1) as cpool, \
             tc.tile_pool(name="big", bufs=1) as big, \
             tc.tile_pool(name="st", bufs=1) as st, \
             tc.tile_pool(name="psA", bufs=1, space="PSUM") as psA, \
             tc.tile_pool(name="psB", bufs=1, space="PSUM") as psB, \
             tc.tile_pool(name="dram", bufs=1, space="DRAM") as dpool:

            # ---------- per-partition constants ----------
            ccol = cpool.tile([128, N_COLS], F32)
            nc.sync.dma_start(out=ccol, in_=ccol_d)

            def col(i):
                return ccol[:, i:i + 1]

            # ---------- generated constant matrices ----------
            pidx = cpool.tile([128, 1], I32, tag="pidx")
            nc.gpsimd.iota(pidx, pattern=[[1, 1]], base=0, channel_multiplier=1)
            fidx = cpool.tile([128, 128], I32, tag="fidx")
            nc.gpsimd.iota(fidx, pattern=[[1, 128]], base=0,
                           channel_multiplier=0)
            pidx_f = cpool.tile([128, 1], F32, tag="pidxf")
            nc.vector.tensor_copy(pidx_f, pidx)
            fidx_f = cpool.tile([128, 128], F32, tag="fidxf")
            nc.vector.tensor_copy(fidx_f, fidx)

            # identity (f16): 1 where f == p
            ident16 = cpool.tile([128, 128], F16, tag="ident16")
            nc.vector.tensor_scalar(ident16, fidx_f, pidx_f, None, OP.is_equal)

            # ind2 (f16): 1/64 where f//64 == p//64
            fdiv = cpool.tile([128, 128], I32, tag="fdiv")
            nc.vector.tensor_scalar(fdiv, fidx, 6, None, OP.arith_shift_right)
            pdiv = cpool.tile([128, 1], I32, tag="pdiv")
            nc.vector.tensor_scalar(pdiv, pidx, 6, None, OP.arith_shift_right)
            fdiv_f = cpool.tile([128, 128], F32, tag="fdivf")
            nc.vector.tensor_copy(fdiv_f, fdiv)
            pdiv_f = cpool.tile([128, 1], F32, tag="pdivf")
            nc.vector.tensor_copy(pdiv_f, pdiv)
            ind2 = cpool.tile([128, 128], F16, tag="ind2")
            nc.vector.tensor_scalar(ind2, fdiv_f, pdiv_f, 1.0 / 64.0,
                                    OP.is_equal, OP.mult)

            # pswap (f16): 1 where f == p^1 (pair swap; rows >=32 are
            # harmless because sintab is 0 there)
            pm2 = cpool.tile([128, 1], I32, tag="pm2")
            nc.vector.tensor_scalar(pm2, pidx, 1, None, OP.bitwise_and)
            tgt = cpool.tile([128, 1], I32, tag="tgt")
            nc.vector.tensor_scalar(tgt, pm2, -2, 1, OP.mult, OP.add)
            nc.vector.tensor_tensor(tgt, tgt, pidx, OP.add)
            tgt_f = cpool.tile([128, 1], F32, tag="tgtf")
            nc.vector.tensor_copy(tgt_f, tgt)
            pswap = cpool.tile([128, 128], F16, tag="pswap")
            nc.vector.tensor_scalar(pswap, fidx_f, tgt_f, None, OP.is_equal)

            # ---------- rope tables (f32 [128, T]) ----------
            costab = cpool.tile([128, T], F32, tag="cost")
            sintab = cpool.tile([128, T], F32, tag="sint")
            halfpi = cpool.tile([128, 1], F32, tag="halfpi")
            nc.vector.memset(halfpi, np.pi / 2.0)
            zerob = cpool.tile([128, 1], F32, tag="zerob")
            nc.vector.memset(zerob, 0.0)
            for cc in range(T // G):
                csl = slice(G * cc, G * (cc + 1))
                ti32 = st.tile([128, G], I32, tag="ti32", bufs=2)
                nc.gpsimd.iota(ti32, pattern=[[1, G]], base=G * cc,
                               channel_multiplier=0)
                th = st.tile([128, G], F32, tag="ropeth", bufs=2)
                nc.vector.tensor_copy(th, ti32)
                nc.vector.tensor_scalar_mul(th, th, col(COL_ANG))
                for tab, shift, bias in ((sintab, 0.0, zerob),
                                         (costab, 0.25, halfpi)):
                    uu = st.tile([128, G], F32, tag="ropeuu")
                    nc.vector.tensor_scalar(uu, th, 1.0 / TWO_PI, shift,
                                            OP.mult, OP.add)
                    ki = st.tile([128, G], I32, tag="ropeki")
                    nc.vector.tensor_copy(ki, uu)   # rounds to nearest
                    kf = st.tile([128, G], F32, tag="ropekf")
                    nc.vector.tensor_copy(kf, ki)
                    red = st.tile([128, G], F32, tag="ropered")
                    nc.vector.scalar_tensor_tensor(red, kf, -TWO_PI, th,
                                                   OP.mult, OP.add)
                    nc.scalar.activation(out=tab[:, csl], in_=red,
                                         func=AF.Sin, bias=bias[:, 0:1])
            nc.vector.tensor_scalar_mul(sintab, sintab, col(COL_SGN))

            # ---------- weights: load + transpose ----------
            wqt = cpool.tile([128, C], F16, tag="wqt")
            wkt = cpool.tile([128, C], F16, tag="wkt")
            wvt = cpool.tile([128, C], F16, tag="wvt")
            wpTh = [cpool.tile([64, C], F16, tag=f"wpT{h}", name=f"wpT{h}")
                    for h in range(2)]
            la_sb = cpool.tile([128, 256], F16, tag="la")
            lb_sb = cpool.tile([32, 128], F16, tag="lb")
            if FP8_W:
                lb_raw = st.tile([32, 128], F8, tag="lbraw")
                nc.sync.dma_start(out=lb_raw, in_=lb_d)
                nc.vector.tensor_copy(lb_sb, lb_raw)
                la_raw = st.tile([128, 256], F8, tag="laraw")
                for j in range(8):
                    nc.sync.dma_start(out=la_raw[:, 32 * j:32 * (j + 1)],
                                      in_=la_d[128 * j:128 * (j + 1), :])
                nc.vector.tensor_copy(la_sb, la_raw)
            else:
                nc.sync.dma_start(out=lb_sb, in_=lb_d)
                for j in range(8):
                    nc.sync.dma_start(out=la_sb[:, 32 * j:32 * (j + 1)],
                                      in_=la_d[128 * j:128 * (j + 1), :])
            for wt, row0 in ((wqt, 0), (wkt, 128), (wvt, 256)):
                wraw = st.tile([128, C], F16, tag="wraw", bufs=2)
                if FP8_W:
                    wraw8 = st.tile([128, C], F8, tag="wraw8", bufs=2)
                    nc.sync.dma_start(out=wraw8,
                                      in_=wqkv_d[row0:row0 + 128, :])
                    nc.vector.tensor_copy(wraw, wraw8)
                else:
                    nc.sync.dma_start(out=wraw,
                                      in_=wqkv_d[row0:row0 + 128, :])
                for j in range(8):
                    tp = psB.tile([128, 128], F16, tag="tp", bufs=1)
                    nc.tensor.transpose(tp, wraw[:, 128 * j:128 * (j + 1)],
                                        ident16)
                    nc.vector.tensor_copy(wt[:, 128 * j:128 * (j + 1)], tp)
            for m in range(8):
                wpraw = st.tile([128, 128], F16, tag="wpraw", bufs=2)
                if FP8_W:
                    wpraw8 = st.tile([128, 128], F8, tag="wpraw8", bufs=2)
                    nc.sync.dma_start(out=wpraw8,
                                      in_=wp_d[128 * m:128 * (m + 1), :])
                    nc.vector.tensor_copy(wpraw, wpraw8)
                else:
                    nc.sync.dma_start(out=wpraw,
                                      in_=wp_d[128 * m:128 * (m + 1), :])
                tp = psB.tile([128, 128], F16, tag="tp", bufs=1)
                nc.tensor.transpose(tp, wpraw, ident16)
                wpscr = st.tile([128, 128], F16, tag="wpscr", bufs=2)
                nc.vector.tensor_copy(wpscr, tp)
                nc.vector.tensor_copy(wpTh[0][:, 128 * m:128 * (m + 1)],
                                      wpscr[0:64, :])
                nc.sync.dma_start(out=wpTh[1][:, 128 * m:128 * (m + 1)],
                                  in_=wpscr[64:128, :])

            # ---------- ingest x/v1: transpose + collectives ----------
            ag_in = dpool.tile([8, 128, G], F16, tag="agin")
            a2a_in = dpool.tile([8, 128, G], F16, tag="a2ain")
            for src_d, dst in ((xs_d, ag_in), (v1s_d, a2a_in)):
                xsb = []
                for a in range(4):
                    xa = st.tile([128, C], F16, tag=f"xsb{a}", name=f"xsb{a}",
                                 bufs=2)
                    if FP8_ACT:
                        xa8 = st.tile([128, C], F8, tag="xa8", bufs=2)
                        nc.sync.dma_start(out=xa8,
                                          in_=src_d[128 * a:128 * (a + 1), :])
                        nc.vector.tensor_copy(xa, xa8)
                    else:
                        nc.sync.dma_start(out=xa,
                                          in_=src_d[128 * a:128 * (a + 1), :])
                    xsb.append(xa)
                for j in range(8):
                    xtj = st.tile([128, G], F16, tag="xtj", bufs=3)
                    for a in range(4):
                        tp = psB.tile([128, 128], F16, tag="tp", bufs=1)
                        nc.tensor.transpose(
                            tp, xsb[a][:, 128 * j:128 * (j + 1)], ident16)
                        nc.vector.tensor_copy(
                            xtj[:, 128 * a:128 * (a + 1)], tp)
                    nc.sync.dma_start(out=dst[j], in_=xtj)
            ag_out = dpool.tile([8, 8, 128, G], F16, tag="agout")
            nc.gpsimd.collective_compute(
                "AllGather", OP.bypass, replica_groups=RG,
                ins=[ag_in.opt()], outs=[ag_out.opt()])
            a2a_out = dpool.tile([8, 128, G], F16, tag="a2aout")
            nc.gpsimd.collective_compute(
                "AllToAll", OP.bypass, replica_groups=RG,
                ins=[a2a_in.opt()], outs=[a2a_out.opt()])

            # ---------- persistent activations ----------
            q_fin = big.tile([128, BT], F16, tag="qfin")
            k_fin = big.tile([128, BT], F16, tag="kfin")
            vaug = [big.tile([128, 32, 65], F16, tag=f"vaug{h}",
                             name=f"vaug{h}") for h in range(2)]
            for h in range(2):
                nc.vector.memset(vaug[h][:, :, 64:65], 1.0)
            yT = [big.tile([64, BT], F16, tag=f"yt{h}", name=f"yt{h}")
                  for h in range(2)]
            carry = big.tile([128, 4], F32, tag="carry")

            # ---------- main per-chunk pipeline ----------
            for g in range(NG):
                first = g % 4 == 0          # batch-boundary chunk
                tcols = slice(G * g, G * (g + 1))
                tsl = slice(G * (g % 4), G * (g % 4 + 1))

                # --- projections ---
                ps_q = psA.tile([128, G], F32, tag="pq")
                ps_k = psA.tile([128, G], F32, tag="pk")
                ps_v = psA.tile([128, G], F32, tag="pv")
                ps_u = psA.tile([32, G], F32, tag="pu")
                for j in range(8):
                    xt = st.tile([128, G], F16, tag="xs", bufs=4)
                    nc.sync.dma_start(out=xt, in_=ag_out[g, j])
                    nc.tensor.matmul(ps_q, wqt[:, 128 * j:128 * (j + 1)], xt,
                                     start=(j == 0), stop=(j == 7))
                    nc.tensor.matmul(ps_k, wkt[:, 128 * j:128 * (j + 1)], xt,
                                     start=(j == 0), stop=(j == 7))
                    nc.tensor.matmul(ps_v, wvt[:, 128 * j:128 * (j + 1)], xt,
                                     start=(j == 0), stop=(j == 7))
                    nc.tensor.matmul(ps_u, la_sb[:, 32 * j:32 * (j + 1)], xt,
                                     start=(j == 0), stop=(j == 7))
                u_sb = st.tile([32, G], F16, tag="us", bufs=2)
                nc.vector.tensor_copy(u_sb, ps_u)
                raw = {}
                for tn, ps in (("q", ps_q), ("k", ps_k)):
                    r = st.tile([128, G], F32, tag=f"raw{tn}", name=f"raw{tn}",
                                bufs=2)
                    nc.vector.tensor_copy(r, ps)
                    raw[tn] = r

                # --- value pipeline ---
                gps = psB.tile([128, G], F32, tag="stat", bufs=2)
                nc.tensor.matmul(gps, lb_sb, u_sb, start=True, stop=True)
                th_t = st.tile([128, G], F32, tag="wA")
                nc.scalar.activation(out=th_t, in_=gps, func=AF.Tanh,
                                     scale=0.5, bias=col(COL_V0H))
                sig = st.tile([128, G], F32, tag="wB")
                nc.vector.tensor_scalar(sig, th_t, 0.5, 0.5, OP.mult, OP.add)
                v1t16 = st.tile([128, G], F16, tag="v1a", bufs=2)
                nc.sync.dma_start(out=v1t16, in_=a2a_out[g])
                v1tile = st.tile([128, G], F32, tag="wC")
                nc.vector.tensor_copy(v1tile, v1t16)
                dd = st.tile([128, G], F32, tag="wD")
                nc.vector.tensor_sub(dd, v1tile, ps_v)
                nc.vector.tensor_mul(dd, dd, sig)
                vg = st.tile([128, G], F32, tag="vg")
                nc.vector.tensor_add(vg, dd, ps_v)

                def shift_mix(src_tile, carry_col, mix_c, om_c, out_tile):
                    # out = om*src + mix*prev(src); prev col0 from carry
                    t1 = st.tile([128, G], F32, tag="t1")
                    nc.vector.tensor_scalar_mul(t1[:, 1:G],
                                                src_tile[:, 0:G - 1], mix_c)
                    if first:
                        nc.vector.tensor_scalar_mul(t1[:, 0:1],
                                                    src_tile[:, 0:1], mix_c)
                    else:
                        nc.vector.tensor_scalar_mul(t1[:, 0:1], carry_col,
                                                    mix_c)
                    nc.vector.scalar_tensor_tensor(out_tile, src_tile, om_c,
                                                   t1, OP.mult, OP.add)
                    nc.vector.tensor_copy(carry_col, src_tile[:, G - 1:G])

                vf = st.tile([128, G], F32, tag="wA2")
                shift_mix(vg, carry[:, 2:3], col(COL_MIXV), col(COL_OMV), vf)
                vf16 = st.tile([128, G], F16, tag="vf16")
                nc.vector.tensor_copy(vf16, vf)
                for i in range(4):
                    tp = psB.tile([128, 128], F16, tag="tp", bufs=1)
                    nc.tensor.transpose(tp, vf16[:, 128 * i:128 * (i + 1)],
                                        ident16)
                    ti = 4 * g + i
                    nc.vector.tensor_copy(vaug[0][:, ti, 0:64], tp[:, 0:64])
                    nc.vector.tensor_copy(vaug[1][:, ti, 0:64], tp[:, 64:128])

                # --- q/k pipeline ---
                for ti, tn in enumerate(("q", "k")):
                    mix_c = col(COL_MIXQ if tn == "q" else COL_MIXK)
                    om_c = col(COL_OMQ if tn == "q" else COL_OMK)
                    w_c = col(COL_WQ if tn == "q" else COL_WK)
                    b_c = col(COL_BQ if tn == "q" else COL_BK)
                    fin = q_fin if tn == "q" else k_fin

                    qs = st.tile([128, G], F32, tag=f"qs{tn}", name=f"qs{tn}",
                                 bufs=2)
                    shift_mix(raw[tn], carry[:, ti:ti + 1], mix_c, om_c, qs)
                    qs16 = st.tile([128, G], F16, tag="qs16", bufs=2)
                    nc.vector.tensor_copy(qs16, qs)
                    ps_mu = psB.tile([128, G], F32, tag="stat", bufs=2)
                    nc.tensor.matmul(ps_mu, ind2, qs16, start=True, stop=True)
                    q2 = st.tile([128, G], F16, tag="wB2")
                    nc.scalar.activation(out=q2, in_=qs, func=AF.Square)
                    ps_m2 = psB.tile([128, G], F32, tag="stat", bufs=2)
                    nc.tensor.matmul(ps_m2, ind2, q2, start=True, stop=True)
                    mu2 = st.tile([128, G], F32, tag="wC2")
                    nc.scalar.activation(out=mu2, in_=ps_mu, func=AF.Square)
                    varb = st.tile([128, G], F32, tag="wD2")
                    nc.vector.scalar_tensor_tensor(varb, ps_m2, LN_EPS, mu2,
                                                   OP.add, OP.subtract)
                    sd = st.tile([128, G], F32, tag="wE2")
                    nc.scalar.activation(out=sd, in_=varb, func=AF.Sqrt)
                    rstd = st.tile([128, G], F32, tag="wF2")
                    nc.vector.reciprocal(rstd, sd)

                    z1 = st.tile([128, G], F32, tag="wE")
                    nc.vector.scalar_tensor_tensor(z1, qs, 0.0, ps_mu,
                                                   OP.bypass, OP.subtract)
                    nc.vector.scalar_tensor_tensor(z1, z1, w_c, rstd,
                                                   OP.mult, OP.mult)
                    z3 = st.tile([128, G], F32, tag=f"z3{tn}", name=f"z3{tn}",
                                 bufs=2)
                    nc.vector.tensor_scalar(z3, z1, b_c, None, OP.add)
                    z316 = st.tile([128, G], F16, tag="z316", bufs=2)
                    nc.vector.tensor_copy(z316, z3)
                    ps_zf = psB.tile([128, G], F32, tag="stat", bufs=2)
                    nc.tensor.matmul(ps_zf, pswap, z316, start=True, stop=True)
                    m1 = st.tile([128, G], F32, tag="wB3")
                    nc.vector.tensor_mul(m1, z3, costab[:, tsl])
                    m2r = st.tile([128, G], F32, tag="wC3")
                    nc.vector.scalar_tensor_tensor(m2r, ps_zf, 0.0,
                                                   sintab[:, tsl],
                                                   OP.bypass, OP.mult)
                    nc.vector.tensor_add(fin[:, tcols], m1, m2r)

            # ---------- attention ----------
            for b in range(B):
                base = T * b
                for h in range(2):
                    hr = slice(64 * h, 64 * (h + 1))
                    for qc in range(4):
                        qsl = slice(base + G * qc, base + G * (qc + 1))
                        y_ps = psB.tile([65, G], F32, tag="bc")
                        nj = 4 * qc + 4
                        for j in range(nj):
                            stp = psB.tile([128, G], F32, tag="stat", bufs=2)
                            ksl = slice(base + 128 * j, base + 128 * (j + 1))
                            nc.tensor.matmul(stp, k_fin[hr, ksl],
                                             q_fin[hr, qsl],
                                             start=True, stop=True)
                            pt = st.tile([128, G], F16, tag="pt", bufs=3)
                            nc.scalar.activation(out=pt, in_=stp, func=AF.Exp,
                                                 scale=SCALE)
                            off = 128 * j - G * qc
                            if off >= 0:
                                nc.gpsimd.affine_select(
                                    out=pt, in_=pt, compare_op=OP.is_ge,
                                    fill=0.0, base=-off, channel_multiplier=-1,
                                    pattern=[[1, G]])
                            nc.tensor.matmul(y_ps, vaug[h][:, 16 * b + j, :],
                                             pt, start=(j == 0),
                                             stop=(j == nj - 1))
                        sscr = dpool.tile([1, G], F32, tag="sscr", bufs=4)
                        srow = st.tile([128, G], F32, tag="srow")
                        nc.scalar.activation(out=srow[64:65, :],
                                             in_=y_ps[64:65, :], func=AF.Copy)
                        nc.sync.dma_start(out=sscr, in_=srow[64:65, :])
                        s_b = st.tile([64, G], F32, tag="sb")
                        nc.sync.dma_start(
                            out=s_b, in_=sscr[0:1, :].broadcast_to([64, G]))
                        rb = st.tile([64, G], F32, tag="rb")
                        nc.vector.reciprocal(rb, s_b)
                        nc.vector.scalar_tensor_tensor(
                            yT[h][:, qsl], y_ps[0:64, :], 0.0, rb,
                            OP.bypass, OP.mult)

            # ---------- c_proj partials + ReduceScatter ----------
            rs_in = dpool.tile([8, G, C], F16, tag="rsin")
            for tt in range(32):
                ts128 = slice(128 * tt, 128 * (tt + 1))
                for half in range(2):
                    csl = slice(512 * half, 512 * (half + 1))
                    ps_o = psB.tile([128, G], F32, tag="stat", bufs=2)
                    nc.tensor.matmul(ps_o, yT[0][:, ts128], wpTh[0][:, csl],
                                     start=True, stop=False)
                    nc.tensor.matmul(ps_o, yT[1][:, ts128], wpTh[1][:, csl],
                                     start=False, stop=True)
                    ob = st.tile([128, G], F16, tag="ob", bufs=3)
                    nc.vector.tensor_copy(ob, ps_o)
                    nc.sync.dma_start(
                        out=rs_in[tt // 4, 128 * (tt % 4):128 * (tt % 4 + 1),
                                  csl],
                        in_=ob)
            rs_out = dpool.tile([G, C], F16, tag="rsout")
            nc.gpsimd.collective_compute(
                "ReduceScatter", OP.add, replica_groups=RG,
                ins=[rs_in.opt()], outs=[rs_out.opt()])
            nc.sync.dma_start(out=out_d, in_=rs_out)

    nc.compile()
    return nc


_CACHE = {}


def _get_nc():
    if "nc" not in _CACHE:
        _CACHE["nc"] = _build()
    return _CACHE["nc"]


def kernel(_results_hook=None, **inputs):
    in_maps = _host_prep(inputs)
    nc = _get_nc()
    res = run_bass_kernel_spmd(nc, in_maps, core_ids=list(range(N_CORES)))
    if _results_hook is not None:
        _results_hook(res)
    y = np.concatenate([res.results[c]["out"] for c in range(N_CORES)],
                       axis=0).astype(np.float32)
    out = np.asarray(inputs["residual"], np.float32).reshape(BT, C) + y
    return out.reshape(B, T, C)
